# revision 7
# baseline (speedup 1.0000x reference)
"""Trainium2 Bass kernel for the CIntegration embedding-lookup module.

reference semantics (all fp32):
    ct    = concat(one_hot(rgap, 32), one_hot(sgap, 32), one_hot(pcount, 64))  # [B,S,128]
    Cct   = W.T[rgap] + W.T[32+sgap] + W.T[64+pcount]                          # [B,S,128]
    theta = vt * Cct
    out   = concat(theta, ct)                                                  # [B,S,256]

Strategy (8 NeuronCores, data-parallel over the batch dim, W replicated):
  The correctness gate is rel-err < 2e-2 of the output scale, which admits
  16-bit I/O end to end (and fp8 for the exact 0/1 one-hot block).  HBM
  bandwidth is the binding constraint (memory regime): f32 I/O costs
  48 MiB/core; bf16 vt/theta + fp8 ct costs 20 MiB.

  Transposed on-chip layout: SBUF partition p = embedding/bin index, free
  dim = token (natural order).  Per 2048-token chunk:
    - b3[p, t] = idx of the bin-block partition p belongs to.  Built by a
      SWDGE SBUF->SBUF DMA whose source AP re-reads the resident idx4
      [4, T] rows with a stride-0 free dim (partition broadcast), so it
      lands in SBUF as bf16 -- no PSUM, no PE involvement.
    - ctT[bin, t] = (b3 == iota): ONE DVE is_equal per chunk in the fast
      all-SBUF 16-bit mode, written straight into the output layout; the
      same bytes are the moving operand of the W-gather matmul.
    - CctT = Wt.T @ ctT: 4 matmuls (stationary Wt bf16 is constant, so the
      weight load amortizes).
    - thetaT = vtT * CctT: one DVE multiply per chunk (PSUM source).
    - ACT engine converts ctT bf16 -> fp8 for the store and issues the
      theta store; Pool issues the broadcast + ct store (SWDGE) so the SP
      HWDGE ring only streams vt loads.
"""

import sys

import numpy as np

try:  # concourse is on sys.path via sitecustomize in the runtime image;
    import concourse  # noqa: F401  # fall back to known locations otherwise
except ImportError:  # pragma: no cover
    for _p in ("/opt/trn_rl_repo", "/root/.axon_site/_ro/trn_rl_repo"):
        if _p not in sys.path:
            sys.path.insert(0, _p)

B, S, EMB = 256, 1024, 128
NUM_RGAP, NUM_SGAP, NUM_PCOUNT = 32, 32, 64
NTOTAL = NUM_RGAP + NUM_SGAP + NUM_PCOUNT  # 128
NCORES = 8
ROWS_PER_CORE = B // NCORES                # 32
T_CORE = ROWS_PER_CORE * S                 # 32768 tokens per core
CHUNK = 2048                               # tokens per chunk
NCHUNK = T_CORE // CHUNK                   # 16
QMM = CHUNK // 512                         # matmuls per chunk (512-col PSUM banks)

CT_FP8 = True                              # one-hot output dtype: fp8e4 vs bf16

_compiled = {}


def _build_program(loop_n=None, ct_fp8=CT_FP8, nchunk=NCHUNK, bufs=4):
    import concourse.bacc as bacc
    import concourse.mybir as mybir
    from concourse import tile

    f32 = mybir.dt.float32
    bf16 = mybir.dt.bfloat16
    fp8 = mybir.dt.float8e4
    Alu = mybir.AluOpType

    nc = bacc.Bacc(None)

    t_core = nchunk * CHUNK
    vt_in = nc.declare_dram_parameter("vt", [nchunk, 128, CHUNK], bf16, isOutput=False)
    idx4_in = nc.declare_dram_parameter("idx4", [4, t_core], bf16, isOutput=False)
    wt_in = nc.declare_dram_parameter("wt", [128, 128], bf16, isOutput=False)
    iota_in = nc.declare_dram_parameter("iota_col", [128, 1], f32, isOutput=False)
    th_ext = nc.declare_dram_parameter(
        "th_out", [nchunk, 128, CHUNK], bf16, isOutput=True
    )
    ct_ext = nc.declare_dram_parameter(
        "ct_out", [nchunk, 128, CHUNK], fp8 if ct_fp8 else bf16, isOutput=True
    )

    with tile.TileContext(nc) as tc:
        with (
            tc.tile_pool(name="consts", bufs=1) as consts,
            tc.tile_pool(name="vt", bufs=bufs) as vtp,
            tc.tile_pool(name="b3", bufs=bufs) as b3p,
            tc.tile_pool(name="ctb", bufs=bufs) as ctp,
            tc.tile_pool(name="th", bufs=bufs) as thp,
            tc.tile_pool(name="ct8", bufs=bufs) as ct8p,
            tc.tile_pool(name="ps_cc", bufs=2, space="PSUM") as pscc,
        ):
            wt = consts.tile([128, 128], bf16, tag="wt")
            iota = consts.tile([128, 1], f32, tag="iota")
            idx4 = consts.tile([4, t_core], bf16, tag="idx4")
            # wt/iota via SWDGE; idx4 gates the broadcast chain so it rides
            # the ACT HWDGE ring (idle until the first store); the SP ring
            # starts streaming vt immediately
            nc.gpsimd.dma_start(out=wt[:, :], in_=wt_in[:, :])
            nc.gpsimd.dma_start(out=iota[:, :], in_=iota_in[:, :])
            nc.scalar.dma_start(out=idx4[:, :], in_=idx4_in[:, :])

            def chunk_body(c):
                t0 = c * CHUNK
                vt_sb = vtp.tile([128, CHUNK], bf16, tag="vt")
                nc.sync.dma_start(out=vt_sb[:, :], in_=vt_in[c])
                # partition-broadcast of the 4 idx rows to 4x32 partitions:
                # SBUF->SBUF SWDGE with a stride-0 middle dim on the source
                b3 = b3p.tile([128, CHUNK], bf16, tag="b3")
                src = idx4[:, t0 : t0 + CHUNK].unsqueeze(1).broadcast_to(
                    [4, 32, CHUNK]
                )
                nc.gpsimd.dma_start(out=b3[:, :], in_=src)
                # ctT[bin, t] = (b3[bin, t] == bin) -- exact 0/1, all-SBUF
                # 16-bit op (fast DVE mode), already in the output layout
                ct_bf = ctp.tile([128, CHUNK], bf16, tag="ctb")
                nc.vector.tensor_scalar(
                    out=ct_bf[:, :],
                    in0=b3[:, :],
                    scalar1=iota[:, :],
                    scalar2=None,
                    op0=Alu.is_equal,
                )
                # CctT[e, t] = sum_bin Wt[bin, e] * ctT[bin, t]
                cc = pscc.tile([128, CHUNK], f32, tag="cc")
                for q in range(QMM):
                    qs = slice(q * 512, (q + 1) * 512)
                    nc.tensor.matmul(
                        cc[:, qs], wt[:, :], ct_bf[:, qs], start=True, stop=True
                    )
                # thetaT = vtT * CctT
                th = thp.tile([128, CHUNK], bf16, tag="th")
                nc.vector.tensor_tensor(
                    out=th[:, :], in0=vt_sb[:, :], in1=cc[:, :], op=Alu.mult
                )
                nc.scalar.dma_start(out=th_ext[c], in_=th[:, :])
                if ct_fp8:
                    ct8 = ct8p.tile([128, CHUNK], fp8, tag="ct8")
                    nc.scalar.copy(out=ct8[:, :], in_=ct_bf[:, :])
                    nc.gpsimd.dma_start(out=ct_ext[c], in_=ct8[:, :])
                else:
                    nc.gpsimd.dma_start(out=ct_ext[c], in_=ct_bf[:, :])

            if loop_n is None:
                for c in range(nchunk):
                    chunk_body(c)
            else:
                with tc.For_i(0, loop_n):
                    for c in range(nchunk):
                        chunk_body(c)

    nc.compile()
    return nc


def _get_compiled(loop_n=None, **kw):
    key = (loop_n, tuple(sorted(kw.items())))
    if key not in _compiled:
        _compiled[key] = _build_program(loop_n, **kw)
    return _compiled[key]


def _host_prep(vt, rgap, sgap, pcount, W):
    import concourse.mybir as mybir

    bf16 = mybir.dt.np(mybir.dt.bfloat16)

    vt = np.asarray(vt)
    W = np.asarray(W, dtype=np.float32)
    rgap = np.asarray(rgap)
    sgap = np.asarray(sgap)
    pcount = np.asarray(pcount)

    wt = np.ascontiguousarray(W.T.astype(bf16))  # [bin, emb]
    iota_col = np.arange(128, dtype=np.float32).reshape(128, 1)

    # idx rows for partition groups 0-31 / 32-63 / 64-95 / 96-127:
    # combined bin indices (int values < 128, exact in bf16)
    pc = NUM_RGAP + NUM_SGAP + pcount
    idx = np.stack([rgap, NUM_RGAP + sgap, pc, pc]).astype(np.float32)  # [4, B, S]

    in_maps = []
    for core in range(NCORES):
        r0 = core * ROWS_PER_CORE
        # vtT per chunk: [emb, token] with tokens in natural order
        vt_c = np.ascontiguousarray(
            vt[r0 : r0 + ROWS_PER_CORE]
            .reshape(NCHUNK, CHUNK, EMB)
            .transpose(0, 2, 1)
            .astype(bf16)
        )
        idx_c = np.ascontiguousarray(
            idx[:, r0 : r0 + ROWS_PER_CORE, :].reshape(4, T_CORE).astype(bf16)
        )
        in_maps.append(
            {"vt": vt_c, "idx4": idx_c, "wt": wt, "iota_col": iota_col}
        )
    return in_maps


def _run(in_maps, trace=False, loop_n=None):
    from concourse.bass_utils import run_bass_kernel_spmd

    nc = _get_compiled(loop_n)
    # transient device wedges (NRT_EXEC_UNIT_UNRECOVERABLE) recover on rerun
    last_err = None
    for _ in range(3):
        try:
            return run_bass_kernel_spmd(nc, in_maps, list(range(NCORES)), trace=trace)
        except Exception as e:  # noqa: BLE001
            if "UNRECOVERABLE" not in str(e) and "UNAVAILABLE" not in str(e):
                raise
            last_err = e
    raise last_err


def kernel(vt, rgap, sgap, pcount, W):
    in_maps = _host_prep(vt, rgap, sgap, pcount, W)
    res = _run(in_maps)
    outs = []
    for core in range(NCORES):
        r = res.results[core]
        th = r["th_out"].astype(np.float32)  # [NCHUNK, 128, CHUNK]
        ct = r["ct_out"].astype(np.float32)
        o = np.stack([th, ct], axis=2)  # [NCHUNK, 128, 2, CHUNK]
        # [c, p, k, t] -> token c*CHUNK + t, feature k*128 + p
        o = o.transpose(0, 3, 2, 1).reshape(ROWS_PER_CORE, S, 2 * EMB)
        outs.append(o)
    return np.ascontiguousarray(np.concatenate(outs, axis=0))


if __name__ == "__main__":
    rng = np.random.default_rng(0)
    vt = rng.standard_normal((B, S, EMB), dtype=np.float32)
    rgap = rng.integers(0, NUM_RGAP, (B, S))
    sgap = rng.integers(0, NUM_SGAP, (B, S))
    pcount = rng.integers(0, NUM_PCOUNT, (B, S))
    W = (rng.standard_normal((EMB, NTOTAL)) * 0.05).astype(np.float32)
    out = kernel(vt, rgap, sgap, pcount, W)
    print(out.shape, out.dtype)


# revision 9
# speedup vs baseline: 4.0019x; 4.0019x over previous
"""Trainium2 Bass kernel for the CIntegration embedding-lookup module.

reference semantics (all fp32):
    ct    = concat(one_hot(rgap, 32), one_hot(sgap, 32), one_hot(pcount, 64))  # [B,S,128]
    Cct   = W.T[rgap] + W.T[32+sgap] + W.T[64+pcount]                          # [B,S,128]
    theta = vt * Cct
    out   = concat(theta, ct)                                                  # [B,S,256]

Strategy (8 NeuronCores, data-parallel over the batch dim, W replicated):
  The correctness gate is rel-err < 2e-2 of the output scale, which admits
  16-bit I/O end to end (and fp8 for the exact 0/1 one-hot block).  DMA
  bandwidth is the binding constraint (memory regime).

  Transposed on-chip layout: SBUF partition p = embedding/bin index, free
  dim = token (natural order).  Per 2048-token chunk:
    - b3[p, t] = combined bin index of the block partition p belongs to,
      host-replicated to 128 rows and streamed from HBM as bf16.  (An
      on-chip broadcast was measured 10x slower: a broadcast-APed DMA
      reads all 128 destination rows from 3 source partitions and
      serializes on their SBUF ports; a PE broadcast lands in PSUM, which
      forces the DVE compare into its slow single-port f32 mode.)
    - ctT[bin, t] = (b3 == iota): ONE DVE is_equal per chunk in the fast
      all-SBUF 16-bit mode, written straight into the output layout; the
      same bytes are the moving operand of the W-gather matmul.
    - CctT = Wt.T @ ctT: 4 matmuls (stationary Wt bf16 constant).
    - thetaT = vtT * CctT: one DVE multiply per chunk (PSUM source).
    - ACT converts ctT bf16 -> fp8 and issues the theta store; Pool issues
      the ct store via SWDGE; the SP HWDGE ring streams the vt + b3 loads.
  Sequential per-chunk emission measures fastest (software-pipelining the
  emission interacted badly with tile-pool rotation on hardware).
"""

import sys

import numpy as np

try:  # concourse is on sys.path via sitecustomize in the runtime image;
    import concourse  # noqa: F401  # fall back to known locations otherwise
except ImportError:  # pragma: no cover
    for _p in ("/opt/trn_rl_repo", "/root/.axon_site/_ro/trn_rl_repo"):
        if _p not in sys.path:
            sys.path.insert(0, _p)

B, S, EMB = 256, 1024, 128
NUM_RGAP, NUM_SGAP, NUM_PCOUNT = 32, 32, 64
NTOTAL = NUM_RGAP + NUM_SGAP + NUM_PCOUNT  # 128
NCORES = 8
ROWS_PER_CORE = B // NCORES                # 32
T_CORE = ROWS_PER_CORE * S                 # 32768 tokens per core
CHUNK = 2048                               # tokens per chunk
NCHUNK = T_CORE // CHUNK                   # 16
QMM = CHUNK // 512                         # matmuls per chunk (512-col PSUM banks)

CT_FP8 = True                              # one-hot output dtype: fp8e4 vs bf16

_compiled = {}


def _build_program(loop_n=None, ct_fp8=CT_FP8, bufs=4, pipelined=False):
    import concourse.bacc as bacc
    import concourse.mybir as mybir
    from concourse import tile

    f32 = mybir.dt.float32
    bf16 = mybir.dt.bfloat16
    fp8 = mybir.dt.float8e4
    Alu = mybir.AluOpType

    nc = bacc.Bacc(None)

    vt_in = nc.declare_dram_parameter("vt", [NCHUNK, 128, CHUNK], bf16, isOutput=False)
    b3_in = nc.declare_dram_parameter(
        "b3rep", [NCHUNK, 128, CHUNK], bf16, isOutput=False
    )
    wt_in = nc.declare_dram_parameter("wt", [128, 128], bf16, isOutput=False)
    iota_in = nc.declare_dram_parameter("iota_col", [128, 1], f32, isOutput=False)
    th_ext = nc.declare_dram_parameter(
        "th_out", [NCHUNK, 128, CHUNK], bf16, isOutput=True
    )
    ct_ext = nc.declare_dram_parameter(
        "ct_out", [NCHUNK, 128, CHUNK], fp8 if ct_fp8 else bf16, isOutput=True
    )

    with tile.TileContext(nc) as tc:
        with (
            tc.tile_pool(name="consts", bufs=1) as consts,
            tc.tile_pool(name="vt", bufs=bufs) as vtp,
            tc.tile_pool(name="b3", bufs=bufs) as b3p,
            tc.tile_pool(name="ctb", bufs=bufs) as ctp,
            tc.tile_pool(name="th", bufs=bufs) as thp,
            tc.tile_pool(name="ct8", bufs=bufs) as ct8p,
            tc.tile_pool(name="ps_cc", bufs=2, space="PSUM") as pscc,
        ):
            wt = consts.tile([128, 128], bf16, tag="wt")
            iota = consts.tile([128, 1], f32, tag="iota")
            nc.gpsimd.dma_start(out=wt[:, :], in_=wt_in[:, :])
            nc.gpsimd.dma_start(out=iota[:, :], in_=iota_in[:, :])

            state = {}

            def front(c):
                """loads + one-hot compare for chunk c"""
                vt_sb = vtp.tile([128, CHUNK], bf16, tag="vt")
                nc.sync.dma_start(out=vt_sb[:, :], in_=vt_in[c])
                b3 = b3p.tile([128, CHUNK], bf16, tag="b3")
                nc.sync.dma_start(out=b3[:, :], in_=b3_in[c])
                ct_bf = ctp.tile([128, CHUNK], bf16, tag="ctb")
                nc.vector.tensor_scalar(
                    out=ct_bf[:, :],
                    in0=b3[:, :],
                    scalar1=iota[:, :],
                    scalar2=None,
                    op0=Alu.is_equal,
                )
                state[c] = (vt_sb, ct_bf)

            def back(c):
                """gather, multiply, convert, stores for chunk c"""
                vt_sb, ct_bf = state.pop(c)
                cc = pscc.tile([128, CHUNK], f32, tag="cc")
                for q in range(QMM):
                    qs = slice(q * 512, (q + 1) * 512)
                    nc.tensor.matmul(
                        cc[:, qs], wt[:, :], ct_bf[:, qs], start=True, stop=True
                    )
                th = thp.tile([128, CHUNK], bf16, tag="th")
                nc.vector.tensor_tensor(
                    out=th[:, :], in0=vt_sb[:, :], in1=cc[:, :], op=Alu.mult
                )
                nc.scalar.dma_start(out=th_ext[c], in_=th[:, :])
                if ct_fp8:
                    ct8 = ct8p.tile([128, CHUNK], fp8, tag="ct8")
                    nc.scalar.copy(out=ct8[:, :], in_=ct_bf[:, :])
                    nc.gpsimd.dma_start(out=ct_ext[c], in_=ct8[:, :])
                else:
                    nc.gpsimd.dma_start(out=ct_ext[c], in_=ct_bf[:, :])

            def body():
                if pipelined:
                    front(0)
                    for c in range(NCHUNK - 1):
                        front(c + 1)
                        back(c)
                    back(NCHUNK - 1)
                else:
                    for c in range(NCHUNK):
                        front(c)
                        back(c)

            if loop_n is None:
                body()
            else:
                with tc.For_i(0, loop_n):
                    body()

    nc.compile()
    return nc


def _get_compiled(loop_n=None, **kw):
    key = (loop_n, tuple(sorted(kw.items())))
    if key not in _compiled:
        _compiled[key] = _build_program(loop_n, **kw)
    return _compiled[key]


_GROUP_ROWS = np.repeat(np.arange(3), [NUM_RGAP, NUM_SGAP, NUM_PCOUNT])  # [128]


def _host_prep(vt, rgap, sgap, pcount, W):
    import concourse.mybir as mybir

    bf16 = mybir.dt.np(mybir.dt.bfloat16)

    vt = np.asarray(vt)
    W = np.asarray(W, dtype=np.float32)
    rgap = np.asarray(rgap)
    sgap = np.asarray(sgap)
    pcount = np.asarray(pcount)

    wt = np.ascontiguousarray(W.T.astype(bf16))  # [bin, emb]
    iota_col = np.arange(128, dtype=np.float32).reshape(128, 1)

    # combined bin indices (int values < 128, exact in bf16)
    idx = np.stack(
        [rgap, NUM_RGAP + sgap, NUM_RGAP + NUM_SGAP + pcount]
    ).astype(np.float32).astype(bf16)  # [3, B, S]

    in_maps = []
    for core in range(NCORES):
        r0 = core * ROWS_PER_CORE
        # vtT per chunk: [emb, token] with tokens in natural order
        vt_c = np.ascontiguousarray(
            vt[r0 : r0 + ROWS_PER_CORE]
            .reshape(NCHUNK, CHUNK, EMB)
            .transpose(0, 2, 1)
            .astype(bf16)
        )
        idx_c = idx[:, r0 : r0 + ROWS_PER_CORE, :].reshape(3, T_CORE)
        # b3 replicated to all 128 partitions: partition p holds the idx
        # stream of the bin-block p belongs to
        b3_c = np.ascontiguousarray(
            idx_c[_GROUP_ROWS]  # [128, T_CORE]
            .reshape(128, NCHUNK, CHUNK)
            .transpose(1, 0, 2)
        )
        in_maps.append(
            {"vt": vt_c, "b3rep": b3_c, "wt": wt, "iota_col": iota_col}
        )
    return in_maps


def _run(in_maps, trace=False, loop_n=None):
    from concourse.bass_utils import run_bass_kernel_spmd

    nc = _get_compiled(loop_n)
    # transient device wedges (NRT_EXEC_UNIT_UNRECOVERABLE) recover on rerun
    last_err = None
    for _ in range(3):
        try:
            return run_bass_kernel_spmd(nc, in_maps, list(range(NCORES)), trace=trace)
        except Exception as e:  # noqa: BLE001
            if "UNRECOVERABLE" not in str(e) and "UNAVAILABLE" not in str(e):
                raise
            last_err = e
    raise last_err


def kernel(vt, rgap, sgap, pcount, W):
    in_maps = _host_prep(vt, rgap, sgap, pcount, W)
    res = _run(in_maps)
    outs = []
    for core in range(NCORES):
        r = res.results[core]
        th = r["th_out"].astype(np.float32)  # [NCHUNK, 128, CHUNK]
        ct = r["ct_out"].astype(np.float32)
        o = np.stack([th, ct], axis=2)  # [NCHUNK, 128, 2, CHUNK]
        # [c, p, k, t] -> token c*CHUNK + t, feature k*128 + p
        o = o.transpose(0, 3, 2, 1).reshape(ROWS_PER_CORE, S, 2 * EMB)
        outs.append(o)
    return np.ascontiguousarray(np.concatenate(outs, axis=0))


if __name__ == "__main__":
    rng = np.random.default_rng(0)
    vt = rng.standard_normal((B, S, EMB), dtype=np.float32)
    rgap = rng.integers(0, NUM_RGAP, (B, S))
    sgap = rng.integers(0, NUM_SGAP, (B, S))
    pcount = rng.integers(0, NUM_PCOUNT, (B, S))
    W = (rng.standard_normal((EMB, NTOTAL)) * 0.05).astype(np.float32)
    out = kernel(vt, rgap, sgap, pcount, W)
    print(out.shape, out.dtype)


# revision 11
# speedup vs baseline: 4.3847x; 1.0956x over previous
"""Trainium2 Bass kernel for the CIntegration embedding-lookup module.

reference semantics (all fp32):
    ct    = concat(one_hot(rgap, 32), one_hot(sgap, 32), one_hot(pcount, 64))  # [B,S,128]
    Cct   = W.T[rgap] + W.T[32+sgap] + W.T[64+pcount]                          # [B,S,128]
    theta = vt * Cct
    out   = concat(theta, ct)                                                  # [B,S,256]

Strategy (8 NeuronCores, data-parallel over the batch dim, W replicated):
  The correctness gate is rel-err < 2e-2 of the output scale, which admits
  16-bit I/O end to end (and fp8 for the exact 0/1 one-hot block).  DMA
  bandwidth is the binding constraint (memory regime).

  Transposed on-chip layout: SBUF partition p = embedding/bin index, free
  dim = token (natural order).  Per 2048-token chunk:
    - b3[p, t] = combined bin index of the block partition p belongs to,
      host-replicated to 128 rows and streamed from HBM as bf16.  (An
      on-chip broadcast was measured 10x slower: a broadcast-APed DMA
      reads all 128 destination rows from 3 source partitions and
      serializes on their SBUF ports; a PE broadcast lands in PSUM, which
      forces the DVE compare into its slow single-port f32 mode.)
    - ctT[bin, t] = (b3 == iota): ONE DVE is_equal per chunk in the fast
      all-SBUF 16-bit mode, written straight into the output layout; the
      same bytes are the moving operand of the W-gather matmul.
    - CctT = Wt.T @ ctT: 4 matmuls (stationary Wt bf16 constant).
    - thetaT = vtT * CctT: one DVE multiply per chunk (PSUM source).
    - ACT converts ctT bf16 -> fp8 and issues the theta store; Pool issues
      the ct store via SWDGE; the SP HWDGE ring streams the vt + b3 loads.
  Sequential per-chunk emission measures fastest (software-pipelining the
  emission interacted badly with tile-pool rotation on hardware).
"""

import sys

import numpy as np

try:  # concourse is on sys.path via sitecustomize in the runtime image;
    import concourse  # noqa: F401  # fall back to known locations otherwise
except ImportError:  # pragma: no cover
    for _p in ("/opt/trn_rl_repo", "/root/.axon_site/_ro/trn_rl_repo"):
        if _p not in sys.path:
            sys.path.insert(0, _p)

B, S, EMB = 256, 1024, 128
NUM_RGAP, NUM_SGAP, NUM_PCOUNT = 32, 32, 64
NTOTAL = NUM_RGAP + NUM_SGAP + NUM_PCOUNT  # 128
NCORES = 8
ROWS_PER_CORE = B // NCORES                # 32
T_CORE = ROWS_PER_CORE * S                 # 32768 tokens per core
CHUNK = 2048                               # tokens per chunk
NCHUNK = T_CORE // CHUNK                   # 16
QMM = CHUNK // 512                         # matmuls per chunk (512-col PSUM banks)

CT_FP8 = True                              # one-hot output dtype: fp8e4 vs bf16

_compiled = {}


def _build_program(loop_n=None, ct_fp8=CT_FP8, bufs=4, pipelined=False, b3_eng="sync", b3_first=False, vb_merge=False):
    import concourse.bacc as bacc
    import concourse.mybir as mybir
    from concourse import tile

    f32 = mybir.dt.float32
    bf16 = mybir.dt.bfloat16
    fp8 = mybir.dt.float8e4
    Alu = mybir.AluOpType

    nc = bacc.Bacc(None)

    if vb_merge:
        vb_in = nc.declare_dram_parameter(
            "vb", [NCHUNK, 128, 2 * CHUNK], bf16, isOutput=False
        )
    else:
        vt_in = nc.declare_dram_parameter(
            "vt", [NCHUNK, 128, CHUNK], bf16, isOutput=False
        )
        b3_in = nc.declare_dram_parameter(
            "b3rep", [NCHUNK, 128, CHUNK], bf16, isOutput=False
        )
    wt_in = nc.declare_dram_parameter("wt", [128, 128], bf16, isOutput=False)
    iota_in = nc.declare_dram_parameter("iota_col", [128, 1], f32, isOutput=False)
    th_ext = nc.declare_dram_parameter(
        "th_out", [NCHUNK, 128, CHUNK], bf16, isOutput=True
    )
    ct_ext = nc.declare_dram_parameter(
        "ct_out", [NCHUNK, 128, CHUNK], fp8 if ct_fp8 else bf16, isOutput=True
    )

    with tile.TileContext(nc) as tc:
        with (
            tc.tile_pool(name="consts", bufs=1) as consts,
            tc.tile_pool(name="vt", bufs=bufs) as vtp,
            tc.tile_pool(name="b3", bufs=bufs) as b3p,
            tc.tile_pool(name="ctb", bufs=bufs) as ctp,
            tc.tile_pool(name="th", bufs=bufs) as thp,
            tc.tile_pool(name="ct8", bufs=bufs) as ct8p,
            tc.tile_pool(name="ps_cc", bufs=2, space="PSUM") as pscc,
        ):
            wt = consts.tile([128, 128], bf16, tag="wt")
            iota = consts.tile([128, 1], f32, tag="iota")
            nc.gpsimd.dma_start(out=wt[:, :], in_=wt_in[:, :])
            nc.gpsimd.dma_start(out=iota[:, :], in_=iota_in[:, :])

            state = {}

            def front(c):
                """loads + one-hot compare for chunk c"""
                beng = getattr(nc, b3_eng)
                if vb_merge:
                    vb = vtp.tile([128, 2 * CHUNK], bf16, tag="vb")
                    nc.sync.dma_start(out=vb[:, :], in_=vb_in[c])
                    vt_sb = vb[:, 0:CHUNK]
                    b3 = vb[:, CHUNK : 2 * CHUNK]
                elif b3_first:
                    b3 = b3p.tile([128, CHUNK], bf16, tag="b3")
                    beng.dma_start(out=b3[:, :], in_=b3_in[c])
                    vt_sb = vtp.tile([128, CHUNK], bf16, tag="vt")
                    nc.sync.dma_start(out=vt_sb[:, :], in_=vt_in[c])
                else:
                    vt_sb = vtp.tile([128, CHUNK], bf16, tag="vt")
                    nc.sync.dma_start(out=vt_sb[:, :], in_=vt_in[c])
                    b3 = b3p.tile([128, CHUNK], bf16, tag="b3")
                    beng.dma_start(out=b3[:, :], in_=b3_in[c])
                ct_bf = ctp.tile([128, CHUNK], bf16, tag="ctb")
                nc.vector.tensor_scalar(
                    out=ct_bf[:, :],
                    in0=b3[:, :],
                    scalar1=iota[:, :],
                    scalar2=None,
                    op0=Alu.is_equal,
                )
                state[c] = (vt_sb, ct_bf)

            def back(c):
                """gather, multiply, convert, stores for chunk c"""
                vt_sb, ct_bf = state.pop(c)
                cc = pscc.tile([128, CHUNK], f32, tag="cc")
                for q in range(QMM):
                    qs = slice(q * 512, (q + 1) * 512)
                    nc.tensor.matmul(
                        cc[:, qs], wt[:, :], ct_bf[:, qs], start=True, stop=True
                    )
                th = thp.tile([128, CHUNK], bf16, tag="th")
                nc.vector.tensor_tensor(
                    out=th[:, :], in0=vt_sb[:, :], in1=cc[:, :], op=Alu.mult
                )
                nc.scalar.dma_start(out=th_ext[c], in_=th[:, :])
                if ct_fp8:
                    ct8 = ct8p.tile([128, CHUNK], fp8, tag="ct8")
                    nc.scalar.copy(out=ct8[:, :], in_=ct_bf[:, :])
                    nc.gpsimd.dma_start(out=ct_ext[c], in_=ct8[:, :])
                else:
                    nc.gpsimd.dma_start(out=ct_ext[c], in_=ct_bf[:, :])

            def body():
                if pipelined:
                    front(0)
                    for c in range(NCHUNK - 1):
                        front(c + 1)
                        back(c)
                    back(NCHUNK - 1)
                else:
                    for c in range(NCHUNK):
                        front(c)
                        back(c)

            if loop_n is None:
                body()
            else:
                with tc.For_i(0, loop_n):
                    body()

    nc.compile()
    return nc


def _get_compiled(loop_n=None, **kw):
    key = (loop_n, tuple(sorted(kw.items())))
    if key not in _compiled:
        _compiled[key] = _build_program(loop_n, **kw)
    return _compiled[key]


_GROUP_ROWS = np.repeat(np.arange(3), [NUM_RGAP, NUM_SGAP, NUM_PCOUNT])  # [128]


def _host_prep(vt, rgap, sgap, pcount, W):
    import concourse.mybir as mybir

    bf16 = mybir.dt.np(mybir.dt.bfloat16)

    vt = np.asarray(vt)
    W = np.asarray(W, dtype=np.float32)
    rgap = np.asarray(rgap)
    sgap = np.asarray(sgap)
    pcount = np.asarray(pcount)

    wt = np.ascontiguousarray(W.T.astype(bf16))  # [bin, emb]
    iota_col = np.arange(128, dtype=np.float32).reshape(128, 1)

    # combined bin indices (int values < 128, exact in bf16)
    idx = np.stack(
        [rgap, NUM_RGAP + sgap, NUM_RGAP + NUM_SGAP + pcount]
    ).astype(np.float32).astype(bf16)  # [3, B, S]

    in_maps = []
    for core in range(NCORES):
        r0 = core * ROWS_PER_CORE
        # vtT per chunk: [emb, token] with tokens in natural order
        vt_c = np.ascontiguousarray(
            vt[r0 : r0 + ROWS_PER_CORE]
            .reshape(NCHUNK, CHUNK, EMB)
            .transpose(0, 2, 1)
            .astype(bf16)
        )
        idx_c = idx[:, r0 : r0 + ROWS_PER_CORE, :].reshape(3, T_CORE)
        # b3 replicated to all 128 partitions: partition p holds the idx
        # stream of the bin-block p belongs to
        b3_c = np.ascontiguousarray(
            idx_c[_GROUP_ROWS]  # [128, T_CORE]
            .reshape(128, NCHUNK, CHUNK)
            .transpose(1, 0, 2)
        )
        in_maps.append(
            {"vt": vt_c, "b3rep": b3_c, "wt": wt, "iota_col": iota_col}
        )
    return in_maps


def _merge_vb(in_maps):
    """convert per-core vt+b3rep inputs to the single merged vb tensor"""
    out = []
    for m in in_maps:
        vb = np.concatenate([m["vt"], m["b3rep"]], axis=2)
        out.append({"vb": np.ascontiguousarray(vb), "wt": m["wt"], "iota_col": m["iota_col"]})
    return out


def _run(in_maps, trace=False, loop_n=None):
    from concourse.bass_utils import run_bass_kernel_spmd

    nc = _get_compiled(loop_n)
    # transient device wedges (NRT_EXEC_UNIT_UNRECOVERABLE) recover on rerun
    last_err = None
    for _ in range(3):
        try:
            return run_bass_kernel_spmd(nc, in_maps, list(range(NCORES)), trace=trace)
        except Exception as e:  # noqa: BLE001
            if "UNRECOVERABLE" not in str(e) and "UNAVAILABLE" not in str(e):
                raise
            last_err = e
    raise last_err


def kernel(vt, rgap, sgap, pcount, W):
    in_maps = _host_prep(vt, rgap, sgap, pcount, W)
    res = _run(in_maps)
    outs = []
    for core in range(NCORES):
        r = res.results[core]
        th = r["th_out"].astype(np.float32)  # [NCHUNK, 128, CHUNK]
        ct = r["ct_out"].astype(np.float32)
        o = np.stack([th, ct], axis=2)  # [NCHUNK, 128, 2, CHUNK]
        # [c, p, k, t] -> token c*CHUNK + t, feature k*128 + p
        o = o.transpose(0, 3, 2, 1).reshape(ROWS_PER_CORE, S, 2 * EMB)
        outs.append(o)
    return np.ascontiguousarray(np.concatenate(outs, axis=0))


if __name__ == "__main__":
    rng = np.random.default_rng(0)
    vt = rng.standard_normal((B, S, EMB), dtype=np.float32)
    rgap = rng.integers(0, NUM_RGAP, (B, S))
    sgap = rng.integers(0, NUM_SGAP, (B, S))
    pcount = rng.integers(0, NUM_PCOUNT, (B, S))
    W = (rng.standard_normal((EMB, NTOTAL)) * 0.05).astype(np.float32)
    out = kernel(vt, rgap, sgap, pcount, W)
    print(out.shape, out.dtype)


# revision 12
# speedup vs baseline: 4.4406x; 1.0128x over previous
"""Trainium2 Bass kernel for the CIntegration embedding-lookup module.

reference semantics (all fp32):
    ct    = concat(one_hot(rgap, 32), one_hot(sgap, 32), one_hot(pcount, 64))  # [B,S,128]
    Cct   = W.T[rgap] + W.T[32+sgap] + W.T[64+pcount]                          # [B,S,128]
    theta = vt * Cct
    out   = concat(theta, ct)                                                  # [B,S,256]

Strategy (8 NeuronCores, data-parallel over the batch dim, W replicated):
  The correctness gate is rel-err < 2e-2 of the output scale, which admits
  16-bit I/O end to end (and fp8 for the exact 0/1 one-hot block).  DMA
  bandwidth is the binding constraint (memory regime).

  Transposed on-chip layout: SBUF partition p = embedding/bin index, free
  dim = token (natural order).  Per 2048-token chunk:
    - b3[p, t] = combined bin index of the block partition p belongs to,
      host-replicated to 128 rows and streamed from HBM as bf16.  (An
      on-chip broadcast was measured 10x slower: a broadcast-APed DMA
      reads all 128 destination rows from 3 source partitions and
      serializes on their SBUF ports; a PE broadcast lands in PSUM, which
      forces the DVE compare into its slow single-port f32 mode.)
    - ctT[bin, t] = (b3 == iota): ONE DVE is_equal per chunk in the fast
      all-SBUF 16-bit mode, written straight into the output layout; the
      same bytes are the moving operand of the W-gather matmul.
    - CctT = Wt.T @ ctT: 4 matmuls (stationary Wt bf16 constant).
    - thetaT = vtT * CctT: one DVE multiply per chunk (PSUM source).
    - ACT converts ctT bf16 -> fp8 and issues the theta store; Pool issues
      the ct store via SWDGE; the SP HWDGE ring streams the vt + b3 loads.
  Sequential per-chunk emission measures fastest (software-pipelining the
  emission interacted badly with tile-pool rotation on hardware).
"""

import sys

import numpy as np

try:  # concourse is on sys.path via sitecustomize in the runtime image;
    import concourse  # noqa: F401  # fall back to known locations otherwise
except ImportError:  # pragma: no cover
    for _p in ("/opt/trn_rl_repo", "/root/.axon_site/_ro/trn_rl_repo"):
        if _p not in sys.path:
            sys.path.insert(0, _p)

B, S, EMB = 256, 1024, 128
NUM_RGAP, NUM_SGAP, NUM_PCOUNT = 32, 32, 64
NTOTAL = NUM_RGAP + NUM_SGAP + NUM_PCOUNT  # 128
NCORES = 8
ROWS_PER_CORE = B // NCORES                # 32
T_CORE = ROWS_PER_CORE * S                 # 32768 tokens per core
CHUNK = 2048                               # tokens per chunk
NCHUNK = T_CORE // CHUNK                   # 16
QMM = CHUNK // 512                         # matmuls per chunk (512-col PSUM banks)

CT_FP8 = True                              # one-hot output dtype: fp8e4 vs bf16

_compiled = {}


def _build_program(loop_n=None, ct_fp8=CT_FP8, bufs=4, pipelined=False, b3_eng="sync", b3_first=False, vb_merge=False, body_rep=1):
    import concourse.bacc as bacc
    import concourse.mybir as mybir
    from concourse import tile

    f32 = mybir.dt.float32
    bf16 = mybir.dt.bfloat16
    fp8 = mybir.dt.float8e4
    Alu = mybir.AluOpType

    nc = bacc.Bacc(None)

    if vb_merge:
        vb_in = nc.declare_dram_parameter(
            "vb", [NCHUNK, 128, 2 * CHUNK], bf16, isOutput=False
        )
    else:
        vt_in = nc.declare_dram_parameter(
            "vt", [NCHUNK, 128, CHUNK], bf16, isOutput=False
        )
        b3_in = nc.declare_dram_parameter(
            "b3rep", [NCHUNK, 128, CHUNK], bf16, isOutput=False
        )
    wt_in = nc.declare_dram_parameter("wt", [128, 128], bf16, isOutput=False)
    iota_in = nc.declare_dram_parameter("iota_col", [128, 1], f32, isOutput=False)
    th_ext = nc.declare_dram_parameter(
        "th_out", [NCHUNK, 128, CHUNK], bf16, isOutput=True
    )
    ct_ext = nc.declare_dram_parameter(
        "ct_out", [NCHUNK, 128, CHUNK], fp8 if ct_fp8 else bf16, isOutput=True
    )

    with tile.TileContext(nc) as tc:
        with (
            tc.tile_pool(name="consts", bufs=1) as consts,
            tc.tile_pool(name="vt", bufs=bufs) as vtp,
            tc.tile_pool(name="b3", bufs=bufs) as b3p,
            tc.tile_pool(name="ctb", bufs=bufs) as ctp,
            tc.tile_pool(name="th", bufs=bufs) as thp,
            tc.tile_pool(name="ct8", bufs=bufs) as ct8p,
            tc.tile_pool(name="ps_cc", bufs=2, space="PSUM") as pscc,
        ):
            wt = consts.tile([128, 128], bf16, tag="wt")
            iota = consts.tile([128, 1], f32, tag="iota")
            nc.gpsimd.dma_start(out=wt[:, :], in_=wt_in[:, :])
            nc.gpsimd.dma_start(out=iota[:, :], in_=iota_in[:, :])

            state = {}

            def front(c):
                """loads + one-hot compare for chunk c"""
                beng = getattr(nc, b3_eng)
                if vb_merge:
                    vb = vtp.tile([128, 2 * CHUNK], bf16, tag="vb")
                    nc.sync.dma_start(out=vb[:, :], in_=vb_in[c])
                    vt_sb = vb[:, 0:CHUNK]
                    b3 = vb[:, CHUNK : 2 * CHUNK]
                elif b3_first:
                    b3 = b3p.tile([128, CHUNK], bf16, tag="b3")
                    beng.dma_start(out=b3[:, :], in_=b3_in[c])
                    vt_sb = vtp.tile([128, CHUNK], bf16, tag="vt")
                    nc.sync.dma_start(out=vt_sb[:, :], in_=vt_in[c])
                else:
                    vt_sb = vtp.tile([128, CHUNK], bf16, tag="vt")
                    nc.sync.dma_start(out=vt_sb[:, :], in_=vt_in[c])
                    b3 = b3p.tile([128, CHUNK], bf16, tag="b3")
                    beng.dma_start(out=b3[:, :], in_=b3_in[c])
                ct_bf = ctp.tile([128, CHUNK], bf16, tag="ctb")
                nc.vector.tensor_scalar(
                    out=ct_bf[:, :],
                    in0=b3[:, :],
                    scalar1=iota[:, :],
                    scalar2=None,
                    op0=Alu.is_equal,
                )
                state[c] = (vt_sb, ct_bf)

            def back(c):
                """gather, multiply, convert, stores for chunk c"""
                vt_sb, ct_bf = state.pop(c)
                cc = pscc.tile([128, CHUNK], f32, tag="cc")
                for q in range(QMM):
                    qs = slice(q * 512, (q + 1) * 512)
                    nc.tensor.matmul(
                        cc[:, qs], wt[:, :], ct_bf[:, qs], start=True, stop=True
                    )
                th = thp.tile([128, CHUNK], bf16, tag="th")
                nc.vector.tensor_tensor(
                    out=th[:, :], in0=vt_sb[:, :], in1=cc[:, :], op=Alu.mult
                )
                nc.scalar.dma_start(out=th_ext[c], in_=th[:, :])
                if ct_fp8:
                    ct8 = ct8p.tile([128, CHUNK], fp8, tag="ct8")
                    nc.scalar.copy(out=ct8[:, :], in_=ct_bf[:, :])
                    nc.gpsimd.dma_start(out=ct_ext[c], in_=ct8[:, :])
                else:
                    nc.gpsimd.dma_start(out=ct_ext[c], in_=ct_bf[:, :])

            def body():
                if pipelined:
                    front(0)
                    for c in range(NCHUNK - 1):
                        front(c + 1)
                        back(c)
                    back(NCHUNK - 1)
                else:
                    for c in range(NCHUNK):
                        front(c)
                        back(c)

            if loop_n is None:
                body()
            else:
                with tc.For_i(0, loop_n):
                    for _ in range(body_rep):
                        body()

    nc.compile()
    return nc


def _get_compiled(loop_n=None, **kw):
    key = (loop_n, tuple(sorted(kw.items())))
    if key not in _compiled:
        _compiled[key] = _build_program(loop_n, **kw)
    return _compiled[key]


_GROUP_ROWS = np.repeat(np.arange(3), [NUM_RGAP, NUM_SGAP, NUM_PCOUNT])  # [128]


def _host_prep(vt, rgap, sgap, pcount, W):
    import concourse.mybir as mybir

    bf16 = mybir.dt.np(mybir.dt.bfloat16)

    vt = np.asarray(vt)
    W = np.asarray(W, dtype=np.float32)
    rgap = np.asarray(rgap)
    sgap = np.asarray(sgap)
    pcount = np.asarray(pcount)

    wt = np.ascontiguousarray(W.T.astype(bf16))  # [bin, emb]
    iota_col = np.arange(128, dtype=np.float32).reshape(128, 1)

    # combined bin indices (int values < 128, exact in bf16)
    idx = np.stack(
        [rgap, NUM_RGAP + sgap, NUM_RGAP + NUM_SGAP + pcount]
    ).astype(np.float32).astype(bf16)  # [3, B, S]

    in_maps = []
    for core in range(NCORES):
        r0 = core * ROWS_PER_CORE
        # vtT per chunk: [emb, token] with tokens in natural order
        vt_c = np.ascontiguousarray(
            vt[r0 : r0 + ROWS_PER_CORE]
            .reshape(NCHUNK, CHUNK, EMB)
            .transpose(0, 2, 1)
            .astype(bf16)
        )
        idx_c = idx[:, r0 : r0 + ROWS_PER_CORE, :].reshape(3, T_CORE)
        # b3 replicated to all 128 partitions: partition p holds the idx
        # stream of the bin-block p belongs to
        b3_c = np.ascontiguousarray(
            idx_c[_GROUP_ROWS]  # [128, T_CORE]
            .reshape(128, NCHUNK, CHUNK)
            .transpose(1, 0, 2)
        )
        in_maps.append(
            {"vt": vt_c, "b3rep": b3_c, "wt": wt, "iota_col": iota_col}
        )
    return in_maps


def _merge_vb(in_maps):
    """convert per-core vt+b3rep inputs to the single merged vb tensor"""
    out = []
    for m in in_maps:
        vb = np.concatenate([m["vt"], m["b3rep"]], axis=2)
        out.append({"vb": np.ascontiguousarray(vb), "wt": m["wt"], "iota_col": m["iota_col"]})
    return out


def _run(in_maps, trace=False, loop_n=None):
    from concourse.bass_utils import run_bass_kernel_spmd

    nc = _get_compiled(loop_n)
    # transient device wedges (NRT_EXEC_UNIT_UNRECOVERABLE) recover on rerun
    last_err = None
    for _ in range(3):
        try:
            return run_bass_kernel_spmd(nc, in_maps, list(range(NCORES)), trace=trace)
        except Exception as e:  # noqa: BLE001
            if "UNRECOVERABLE" not in str(e) and "UNAVAILABLE" not in str(e):
                raise
            last_err = e
    raise last_err


def kernel(vt, rgap, sgap, pcount, W):
    in_maps = _host_prep(vt, rgap, sgap, pcount, W)
    res = _run(in_maps)
    outs = []
    for core in range(NCORES):
        r = res.results[core]
        th = r["th_out"].astype(np.float32)  # [NCHUNK, 128, CHUNK]
        ct = r["ct_out"].astype(np.float32)
        o = np.stack([th, ct], axis=2)  # [NCHUNK, 128, 2, CHUNK]
        # [c, p, k, t] -> token c*CHUNK + t, feature k*128 + p
        o = o.transpose(0, 3, 2, 1).reshape(ROWS_PER_CORE, S, 2 * EMB)
        outs.append(o)
    return np.ascontiguousarray(np.concatenate(outs, axis=0))


if __name__ == "__main__":
    rng = np.random.default_rng(0)
    vt = rng.standard_normal((B, S, EMB), dtype=np.float32)
    rgap = rng.integers(0, NUM_RGAP, (B, S))
    sgap = rng.integers(0, NUM_SGAP, (B, S))
    pcount = rng.integers(0, NUM_PCOUNT, (B, S))
    W = (rng.standard_normal((EMB, NTOTAL)) * 0.05).astype(np.float32)
    out = kernel(vt, rgap, sgap, pcount, W)
    print(out.shape, out.dtype)
